# revision 48
# baseline (speedup 1.0000x reference)
"""Trainium2 Bass kernel for BoundaryConvLayer GNN message passing.

Strategy (8 NeuronCores, no collectives):
  - Nodes are assigned to 8 cores x (windows of 128 node slots), load-balanced
    by in-degree so every window has ~equal incoming-edge count.
  - x is replicated (bf16) on every core as the gather table; each core
    DMA-gathers x[src] rows (256B) for the edges of its own windows and
    reduces them on the TensorEngine: agg_window += S_tile.T @ G_tile where
    S is a one-hot (edge -> dst slot) matrix built on the VectorEngine with
    an is_equal compare against an iota constant.
  - dma_gather descriptor generation is round-robined over the 4 SWDGE
    queues (queue q runs on Q7 core pair q) to parallelize the Q7 work.
  - The per-node MLPs (rate / rob_bound / fc) run in bf16 with
    transposed-activation matmul patterns; LayerNorms via bn_stats in f32.
  - Output is written per-core and inverse-permuted on the host.
"""

import sys

sys.path.insert(0, "/opt/trn_rl_repo")

import heapq

import ml_dtypes
import numpy as np

import bass_rust as _bass_rust

from concourse import bacc, bass, tile
from concourse.bass_utils import run_bass_kernel_spmd
from concourse.hw_specs import get_activation_tables

mybir = bass.mybir
f32 = mybir.dt.float32
bf16 = mybir.dt.bfloat16
u8 = mybir.dt.uint8
i16 = mybir.dt.int16

P = 128
N_CORES = 8
TRACE = False
LAST_EXEC_TIME_NS = None
# gather-index chunk boundaries (int16 idx limit 32767 per chunk); sized so
# the per-(window,chunk) edge-count maxima quantize to fewer 128-edge tiles
CHUNK_BOUNDS = [0, 27000, 54000, 81000, 100000]
N_CHUNKS = 4
STRIP_TRAILING_PADS = False
EPS = 1e-4
LN_EPS = 1e-5


# ----------------------------------------------------------------------------
# host-side planning
# ----------------------------------------------------------------------------

def _balanced_assignment(indeg, n_slots):
    """Assign node ids (len(indeg) <= n_slots) to n_slots//128 windows of 128
    slots each, minimizing max window edge-load. Returns slot_of_node."""
    n_win = n_slots // P
    n = len(indeg)
    order = np.argsort(-indeg, kind="stable")
    slot_of_node = np.empty(n_slots, dtype=np.int64)
    # heap of (load, win); counts per window
    heap = [(0.0, w) for w in range(n_win)]
    heapq.heapify(heap)
    counts = np.zeros(n_win, dtype=np.int64)
    # assign real nodes by descending degree, then dummies
    ids = np.concatenate([order, np.arange(n, n_slots)])
    degs = np.concatenate([indeg[order], np.zeros(n_slots - n, dtype=indeg.dtype)])
    for i in range(n_slots):
        while True:
            load, w = heapq.heappop(heap)
            if counts[w] < P:
                break
        slot_of_node[ids[i]] = w * P + counts[w]
        counts[w] += 1
        heapq.heappush(heap, (load + float(degs[i]), w))
    return slot_of_node


def _plan(x, edge_index, degree):
    N, D = x.shape
    assert D == P
    E = edge_index.shape[1]
    spc = -(-N // (N_CORES * P)) * P          # node slots per core
    n_win = spc // P                           # windows per core
    n_slots = spc * N_CORES
    n_chunks = N_CHUNKS
    bounds = np.array(CHUNK_BOUNDS, dtype=np.int64)
    assert bounds[-1] >= N and (np.diff(bounds) <= 32768).all()

    src = np.asarray(edge_index[0], dtype=np.int64)
    dst = np.asarray(edge_index[1], dtype=np.int64)
    indeg = np.bincount(dst, minlength=N)
    slot_of_node = _balanced_assignment(indeg, n_slots)
    node_of_slot = np.empty(n_slots, dtype=np.int64)
    node_of_slot[slot_of_node] = np.arange(n_slots)

    gslot = slot_of_node[dst]
    core = gslot // spc
    pos = gslot % spc
    gw = pos >> 7                    # window within core
    lane = pos & 127                 # dst slot within window
    chunk = np.searchsorted(bounds, src, side="right") - 1
    srcloc = (src - bounds[chunk]).astype(np.int16)

    # group edges by (core, gw, chunk); compute per-group ranks
    key = (core * n_win + gw) * n_chunks + chunk
    sort_idx = np.argsort(key, kind="stable")
    key_s = key[sort_idx]
    first = np.ones(E, dtype=bool)
    first[1:] = key_s[1:] != key_s[:-1]
    starts = np.flatnonzero(first)
    group_of = np.cumsum(first) - 1
    rank = np.arange(E) - starts[group_of]

    counts = np.bincount(key, minlength=N_CORES * n_win * n_chunks)
    counts = counts.reshape(N_CORES, n_win, n_chunks)
    # tiles per chunk (shared across cores/windows so the program is SPMD)
    TC = np.maximum(1, -(-counts.max(axis=(0, 1)) // P))   # [n_chunks]
    nt = int(TC.sum())
    cumTC = np.concatenate([[0], np.cumsum(TC)])           # [n_chunks+1]

    # super-tile structure: groups of up to 4 windows
    supers = []
    w0 = 0
    while w0 < n_win:
        W = min(4, n_win - w0)
        supers.append((w0, W))
        w0 += W
    super_of_win = np.zeros(n_win, dtype=np.int64)
    Wdim = np.zeros(len(supers), dtype=np.int64)
    super_base = np.zeros(len(supers), dtype=np.int64)   # slot offset in stream
    off = 0
    for si, (sw0, W) in enumerate(supers):
        super_of_win[sw0:sw0 + W] = si
        Wdim[si] = W
        super_base[si] = off
        off += W * nt * P
    total_slots = off
    assert total_slots == n_win * nt * P
    n_tiles_tot = n_win * nt

    # per-edge stream position (within its core's stream)
    # layout within a super: [chunk c: [window wl: TC[c]*128 slots]]
    e_si = super_of_win[gw]
    e_wl = gw - np.array([s[0] for s in supers])[e_si]
    e_W = Wdim[e_si]
    e_pos = (super_base[e_si]
             + e_W * P * cumTC[chunk]
             + e_wl * int(P) * TC[chunk])
    e_pos_sorted = e_pos[sort_idx] + rank
    assert (rank < TC[chunk[sort_idx]] * P).all(), "tile capacity exceeded"

    # build per-core streams
    idx_streams = np.zeros((N_CORES, total_slots), dtype=np.int16)
    slots_2d = np.full((N_CORES, n_tiles_tot, P), 255, dtype=np.uint8)
    core_s = core[sort_idx]
    srcloc_s = srcloc[sort_idx]
    lane_s = lane[sort_idx]
    gw_s = gw[sort_idx]
    chunk_s = chunk[sort_idx]
    rank_s = rank
    e_wl_s = e_wl[sort_idx]
    e_W_s = e_W[sort_idx]
    for c in range(N_CORES):
        m = core_s == c
        idx_streams[c, e_pos_sorted[m]] = srcloc_s[m]
        # chunk-major tile columns within each super: the device builds one
        # one-hot S per chunk from a contiguous slots_t slice
        stb = (gw_s[m] - e_wl_s[m]) * nt
        col = (stb + cumTC[chunk_s[m]] * e_W_s[m]
               + e_wl_s[m] * TC[chunk_s[m]] + (rank_s[m] >> 7))
        slots_2d[c, col, rank_s[m] & 127] = lane_s[m]
        if STRIP_TRAILING_PADS:
            # trailing pad slots of each (super, chunk) gather call -> -1 so
            # the Q7 ucode's trailing-negative strip skips their descriptors
            used = np.zeros(total_slots + 1, dtype=bool)
            used[e_pos_sorted[m]] = True
            for si in range(len(supers)):
                W = int(Wdim[si])
                for ch in range(n_chunks):
                    b0 = int(super_base[si]) + W * P * cumTC[ch]
                    b1 = b0 + W * P * (cumTC[ch + 1] - cumTC[ch])
                    blk = used[b0:b1]
                    nz = np.flatnonzero(blk)
                    last = (nz[-1] + 1) if len(nz) else 0
                    idx_streams[c, b0 + last:b1] = -1

    plan = dict(
        N=N, D=D, E=E, spc=spc, n_win=n_win, n_chunks=n_chunks,
        TC=[int(t) for t in TC], nt=nt, cumTC=[int(t) for t in cumTC],
        supers=supers, total_slots=total_slots, n_tiles_tot=n_tiles_tot,
        slot_of_node=slot_of_node, node_of_slot=node_of_slot,
        idx_streams=idx_streams, slots_2d=slots_2d,
    )
    return plan


# ----------------------------------------------------------------------------
# device program
# ----------------------------------------------------------------------------

class _Bacc(bacc.Bacc):
    """Bacc with an activation-table chooser biased to the exp+ln+identity
    table (act_info idx 6) so exp/ln pairs don't thrash ACT_TABLE_LOADs.
    The per-instruction ids are positions in the passed list, so tables are
    emptied (not removed) to keep ids aligned with act_info.json."""

    def insert_act_table_loads(self):
        has_activation = any(
            isinstance(i, mybir.InstActivation)
            for b in self.main_func.blocks
            for i in b.instructions
        )
        if not has_activation:
            return
        tables = list(get_activation_tables(self.m.arch).items())
        tables = [(n, (s if i >= 6 else set())) for i, (n, s) in enumerate(tables)]
        _bass_rust.insert_act_table_loads(self, tables)


def _build_program(plan, flags):
    n_win, n_chunks = plan["n_win"], plan["n_chunks"]
    TC, nt, cumTC = plan["TC"], plan["nt"], plan["cumTC"]
    supers = plan["supers"]
    spc = plan["spc"]
    N = plan["N"]
    total_slots = plan["total_slots"]
    n_tiles_tot = plan["n_tiles_tot"]
    H = flags["H"]
    nH = H // P

    nc = _Bacc("TRN2", target_bir_lowering=False, num_swdge_queues=4)

    xg_d = nc.dram_tensor("xg", [N, P], bf16, kind="ExternalInput")
    xt_d = nc.dram_tensor("xt", [P, spc], bf16, kind="ExternalInput")
    xf_d = nc.dram_tensor("xf", [spc, P], f32, kind="ExternalInput")
    deg_d = nc.dram_tensor("deg", [P, n_win], f32, kind="ExternalInput")
    idx_d = nc.dram_tensor("idx", [P, total_slots // 16], i16, kind="ExternalInput")
    slots_d = nc.dram_tensor("slots", [P, n_tiles_tot], u8, kind="ExternalInput")
    iota_d = nc.dram_tensor("iota", [P, P], u8, kind="ExternalInput")
    Wr_d = nc.dram_tensor("Wr", [P, P], bf16, kind="ExternalInput")
    Wb1_d = nc.dram_tensor("Wb1", [P, H], bf16, kind="ExternalInput")
    W1_d = nc.dram_tensor("W1", [P, H], bf16, kind="ExternalInput")
    Wb2c_d = nc.dram_tensor("Wb2c", [P, nH, P], bf16, kind="ExternalInput")
    W2c_d = nc.dram_tensor("W2c", [P, nH, P], bf16, kind="ExternalInput")
    bb1c_d = nc.dram_tensor("bb1c", [P, nH], f32, kind="ExternalInput")
    b1c_d = nc.dram_tensor("b1c", [P, nH], f32, kind="ExternalInput")
    ident_d = nc.dram_tensor("ident", [P, P], bf16, kind="ExternalInput")
    bcast_names = [n for n in ("brb", "bb2b", "grbb", "brbb", "gnb", "bnb")
                   if flags[n]]
    bcast_d = {n: nc.dram_tensor(n, [P, P], f32, kind="ExternalInput")
               for n in bcast_names}
    out_d = nc.dram_tensor("out", [spc, P], f32, kind="ExternalOutput")

    AT = mybir.ActivationFunctionType
    OP = mybir.AluOpType

    with tile.TileContext(nc) as tc:
        with tc.tile_pool(name="const", bufs=1) as cp, \
             tc.tile_pool(name="stream", bufs=3) as sp, \
             tc.tile_pool(name="gat", bufs=3) as gp, \
             tc.tile_pool(name="sS", bufs=2) as ssp, \
             tc.tile_pool(name="work", bufs=2) as wp, \
             tc.tile_pool(name="tiny", bufs=6) as tp, \
             tc.tile_pool(name="ps_big", bufs=2, space="PSUM") as ps_big, \
             tc.tile_pool(name="ps_z", bufs=1, space="PSUM") as ps_z, \
             tc.tile_pool(name="ps_agg", bufs=2, space="PSUM") as ps_agg, \
             tc.tile_pool(name="ps_r", bufs=1, space="PSUM") as ps_r, \
             tc.tile_pool(name="ps_ht", bufs=1, space="PSUM") as ps_ht, \
             tc.tile_pool(name="ps_out", bufs=1, space="PSUM") as ps_out:

            # ---- constants
            def cload(dram, shape, dtype, tag):
                t = cp.tile(shape, dtype, tag=tag)
                nc.sync.dma_start(out=t[:], in_=dram[:])
                return t

            eps_t = cp.tile([P, 1], f32, tag="eps")
            nc.vector.memset(eps_t[:], LN_EPS)
            one_t = cp.tile([P, 1], f32, tag="one")
            nc.vector.memset(one_t[:], 1.0)
            onep_t = cp.tile([P, 1], f32, tag="onep")
            nc.vector.memset(onep_t[:], 1.0 + EPS)

            def bc1(t, shape):
                return t[:].rearrange("p (o f) -> p o f", o=1, f=1).broadcast_to(shape)

            Wr_t = cload(Wr_d, [P, P], bf16, "Wr")
            Wb1_t = cload(Wb1_d, [P, H], bf16, "Wb1")
            W1_t = cload(W1_d, [P, H], bf16, "W1")
            Wb2c_t = cload(Wb2c_d, [P, nH, P], bf16, "Wb2c")
            W2c_t = cload(W2c_d, [P, nH, P], bf16, "W2c")
            bb1c_t = cload(bb1c_d, [P, nH], f32, "bb1c")
            b1c_t = cload(b1c_d, [P, nH], f32, "b1c")
            ident_t = cload(ident_d, [P, P], bf16, "ident")
            iota_t = cload(iota_d, [P, P], u8, "iota")
            deg_t = cload(deg_d, [P, n_win], f32, "deg")
            bcast_t = {n: cload(bcast_d[n], [P, P], f32, n) for n in bcast_names}

            stream_col = 0      # idx_d column offset (int16 cols)
            tile_col = 0        # slots_d column offset
            for si, (w0, W) in enumerate(supers):
                R = W * P
                sup_slots = W * nt * P         # gather slots this super
                # ---- streams
                xt_s = sp.tile([P, R], bf16, tag="xt")
                nc.sync.dma_start(out=xt_s[:], in_=xt_d[:, w0 * P: w0 * P + R])
                xf_s = sp.tile([P, W, P], f32, tag="xf")
                nc.sync.dma_start(
                    out=xf_s[:],
                    in_=xf_d[w0 * P: w0 * P + R, :].rearrange(
                        "(w p) f -> p w f", p=P))
                idx_t = sp.tile([P, sup_slots // 16], i16, tag="idx")
                nc.sync.dma_start(
                    out=idx_t[:],
                    in_=idx_d[:, stream_col: stream_col + sup_slots // 16])
                slots_t = sp.tile([P, W * nt], u8, tag="slots")
                nc.sync.dma_start(
                    out=slots_t[:],
                    in_=slots_d[:, tile_col: tile_col + W * nt])

                # ---- gathers (one per chunk; round-robin over SWDGE queues)
                Gs = []
                Ss = []
                for c in range(n_chunks):
                    cap_c = W * TC[c] * P
                    G = gp.tile([P, W * TC[c], P], bf16, tag=f"G{c}")
                    lo = CHUNK_BOUNDS[c]
                    hi = min(N, CHUNK_BOUNDS[c + 1])
                    base = W * P * cumTC[c]
                    nc.gpsimd.dma_gather(
                        out_ap=G[:],
                        in_ap=xg_d[lo:hi, :],
                        idxs_ap=idx_t[:, base // 16: (base + cap_c) // 16],
                        num_idxs=cap_c, num_idxs_reg=cap_c, elem_size=P,
                        single_packet=False, queue_num=c % 4)
                    Gs.append(G)
                    # one-hot S for this chunk (contiguous slots_t slice)
                    S = ssp.tile([P, W * TC[c], P], bf16, tag=f"S{c}")
                    sb = slots_t[:, W * cumTC[c]: W * cumTC[c + 1]
                                 ].broadcast_to([P, W * TC[c], P])
                    ib = iota_t[:].rearrange(
                        "p (o f) -> p o f", o=1).broadcast_to(
                            [P, W * TC[c], P])
                    nc.vector.tensor_tensor(out=S[:], in0=sb, in1=ib,
                                            op=OP.is_equal)
                    Ss.append(S)

                # ---- rob_bound layer 1 (A-pattern):
                # g1 = softplus(u + bb1) = ln(exp(u + bb1) + 1); exp and ln
                # live in the same HW activation table (no table swaps).
                g1T = wp.tile([P, nH, R], bf16, tag="g1T")
                for cH in range(nH):
                    pb = ps_big.tile([P, R], f32, tag="bigA")
                    nc.tensor.matmul(pb[:], Wb1_t[:, cH * P:(cH + 1) * P],
                                     xt_s[:], start=True, stop=True)
                    t1 = wp.tile([P, R], f32, tag="sp1")
                    nc.scalar.activation(t1[:], pb[:], AT.Exp,
                                         bias=bb1c_t[:, cH:cH + 1], scale=1.0)
                    nc.scalar.activation(g1T[:, cH, :], t1[:], AT.Ln,
                                         bias=1.0, scale=1.0)

                # ---- rob_bound layer 2 (B-pattern) -> psum_z[rows, feat]
                pz = ps_z.tile([P, W, P], f32, tag="z")
                for wl in range(W):
                    for cH in range(nH):
                        nc.tensor.matmul(
                            pz[:, wl, :],
                            g1T[:, cH, wl * P:(wl + 1) * P],
                            Wb2c_t[:, cH, :],
                            start=(cH == 0), stop=(cH == nH - 1))

                # optional bb2: LN input z'' = z + bb2
                z_in = pz
                if flags["bb2b"]:
                    z_sb = wp.tile([P, W, P], f32, tag="z_sb")
                    bb2_b = bcast_t["bb2b"][:].rearrange(
                        "p (o f) -> p o f", o=1).broadcast_to([P, W, P])
                    nc.vector.tensor_tensor(out=z_sb[:], in0=pz[:], in1=bb2_b,
                                            op=OP.add)
                    z_in = z_sb

                # ---- LN(gamma) on z  (batched stats; vector apply)
                mv_z = tp.tile([P, W, 2], f32, tag="mv_z")
                for wl in range(W):
                    stats_z = tp.tile([P, 6], f32, tag="stats_z")
                    nc.vector.bn_stats(stats_z[:], z_in[:, wl, :])
                    nc.vector.bn_aggr(mv_z[:, wl, :], stats_z[:])
                # rsqrt(v + eps) = exp(-0.5 * ln(v + eps)) -- same act table
                lnv_z = tp.tile([P, W], f32, tag="lnv_z")
                nc.scalar.activation(
                    lnv_z[:], mv_z[:, :, 1:2].rearrange("p w o -> p (w o)"),
                    AT.Ln, bias=eps_t[:, 0:1], scale=1.0)
                rs_z = tp.tile([P, W], f32, tag="rs_z")
                nc.scalar.activation(rs_z[:], lnv_z[:], AT.Exp,
                                     bias=0.0, scale=-0.5)
                mb_z = tp.tile([P, W], f32, tag="mb_z")
                nc.vector.scalar_tensor_tensor(
                    out=mb_z[:], in0=mv_z[:, :, 0:1].rearrange("p w o -> p (w o)"),
                    scalar=-1.0, in1=rs_z[:], op0=OP.mult, op1=OP.mult)
                gamma = wp.tile([P, W, P], f32, tag="gamma")
                rs_zb = rs_z[:].rearrange("p (w o) -> p w o", o=1).broadcast_to(
                    [P, W, P])
                mb_zb = mb_z[:].rearrange("p (w o) -> p w o", o=1).broadcast_to(
                    [P, W, P])
                nc.vector.tensor_tensor(out=gamma[:], in0=z_in[:], in1=rs_zb,
                                        op=OP.mult)
                nc.vector.tensor_tensor(out=gamma[:], in0=gamma[:], in1=mb_zb,
                                        op=OP.add)
                if flags["grbb"]:
                    g_b = bcast_t["grbb"][:].rearrange(
                        "p (o f) -> p o f", o=1).broadcast_to([P, W, P])
                    nc.vector.tensor_tensor(out=gamma[:], in0=gamma[:], in1=g_b,
                                            op=OP.mult)
                if flags["brbb"]:
                    b_b = bcast_t["brbb"][:].rearrange(
                        "p (o f) -> p o f", o=1).broadcast_to([P, W, P])
                    nc.vector.tensor_tensor(out=gamma[:], in0=gamma[:], in1=b_b,
                                            op=OP.add)

                # ---- rate = softplus(x @ Wr + br)   [rows, feat]
                pr = ps_r.tile([P, W, P], f32, tag="r")
                for wl in range(W):
                    nc.tensor.matmul(pr[:, wl, :], xt_s[:, wl * P:(wl + 1) * P],
                                     Wr_t[:], start=True, stop=True)
                rate = wp.tile([P, W, P], f32, tag="rate")
                if flags["brb"]:
                    br_b = bcast_t["brb"][:].rearrange(
                        "p (o f) -> p o f", o=1).broadcast_to([P, W, P])
                    nc.vector.tensor_tensor(out=rate[:], in0=pr[:], in1=br_b,
                                            op=OP.add)
                    nc.scalar.activation(rate[:], rate[:], AT.Exp,
                                         bias=0.0, scale=1.0)
                else:
                    nc.scalar.activation(rate[:], pr[:], AT.Exp,
                                         bias=0.0, scale=1.0)
                nc.scalar.activation(rate[:], rate[:], AT.Ln,
                                     bias=1.0, scale=1.0)

                # ---- agg: one-hot matmul accumulation per window.
                # chunk-outer order so each G/S buffer is released as soon
                # as its chunk is consumed (unblocks the next gathers).
                pagg = ps_agg.tile([P, W, P], f32, tag="agg")
                for c in range(n_chunks):
                    for wl in range(W):
                        for j in range(TC[c]):
                            nc.tensor.matmul(
                                pagg[:, wl, :],
                                Ss[c][:, wl * TC[c] + j, :],
                                Gs[c][:, wl * TC[c] + j, :],
                                start=(c == 0 and j == 0),
                                stop=(c == n_chunks - 1 and j == TC[c] - 1))

                # ---- h = (rate*agg + gamma) / (1 + rate*deg + EPS)
                num = wp.tile([P, W, P], f32, tag="num")
                nc.vector.tensor_tensor(out=num[:], in0=rate[:], in1=pagg[:],
                                        op=OP.mult)
                nc.vector.tensor_tensor(out=num[:], in0=num[:], in1=gamma[:],
                                        op=OP.add)
                den = wp.tile([P, W, P], f32, tag="den")
                deg_b = deg_t[:, w0:w0 + W].rearrange(
                    "p (w o) -> p w o", o=1).broadcast_to([P, W, P])
                nc.vector.tensor_tensor(out=den[:], in0=rate[:], in1=deg_b,
                                        op=OP.mult)
                nc.vector.tensor_tensor(out=den[:], in0=den[:],
                                        in1=bc1(onep_t, [P, W, P]), op=OP.add)
                rcp = wp.tile([P, W, P], f32, tag="rcp")
                nc.vector.reciprocal_approx_fast(out=rcp[:], in_=den[:])
                h_bf = wp.tile([P, W, P], bf16, tag="h_bf")
                nc.vector.tensor_tensor(out=h_bf[:], in0=num[:], in1=rcp[:],
                                        op=OP.mult)

                # ---- hT via PE transpose
                pht = ps_ht.tile([P, W, P], bf16, tag="ht")
                for wl in range(W):
                    nc.tensor.transpose(pht[:, wl, :], h_bf[:, wl, :], ident_t[:])
                hT = wp.tile([P, R], bf16, tag="hT")
                nc.scalar.activation(hT[:], pht[:].rearrange("p w f -> p (w f)"),
                                     AT.Identity, bias=0.0, scale=1.0)

                # ---- fc layer 1 (A-pattern on hT) + gelu
                g2T = wp.tile([P, nH, R], bf16, tag="g2T")
                for cH in range(nH):
                    pf = ps_big.tile([P, R], f32, tag="bigA")
                    nc.tensor.matmul(pf[:], W1_t[:, cH * P:(cH + 1) * P],
                                     hT[:], start=True, stop=True)
                    nc.scalar.activation(g2T[:, cH, :], pf[:], AT.Gelu,
                                         bias=b1c_t[:, cH:cH + 1], scale=1.0)

                # ---- fc layer 2 (B-pattern) -> psum_out[rows, feat]
                po = ps_out.tile([P, W, P], f32, tag="o")
                for wl in range(W):
                    for cH in range(nH):
                        nc.tensor.matmul(
                            po[:, wl, :],
                            g2T[:, cH, wl * P:(wl + 1) * P],
                            W2c_t[:, cH, :],
                            start=(cH == 0), stop=(cH == nH - 1))

                # ---- x_res = LN(x) (f32) ; out = psum_out + x_res (+ bnb)
                mv_x = tp.tile([P, W, 2], f32, tag="mv_x")
                for wl in range(W):
                    stats_x = tp.tile([P, 6], f32, tag="stats_x")
                    nc.vector.bn_stats(stats_x[:], xf_s[:, wl, :])
                    nc.vector.bn_aggr(mv_x[:, wl, :], stats_x[:])
                lnv_x = tp.tile([P, W], f32, tag="lnv_x")
                nc.scalar.activation(
                    lnv_x[:], mv_x[:, :, 1:2].rearrange("p w o -> p (w o)"),
                    AT.Ln, bias=eps_t[:, 0:1], scale=1.0)
                rs_x = tp.tile([P, W], f32, tag="rs_x")
                nc.scalar.activation(rs_x[:], lnv_x[:], AT.Exp,
                                     bias=0.0, scale=-0.5)
                mb_x = tp.tile([P, W], f32, tag="mb_x")
                nc.vector.scalar_tensor_tensor(
                    out=mb_x[:], in0=mv_x[:, :, 0:1].rearrange("p w o -> p (w o)"),
                    scalar=-1.0, in1=rs_x[:], op0=OP.mult, op1=OP.mult)
                xres = wp.tile([P, W, P], f32, tag="xres")
                rs_xb = rs_x[:].rearrange("p (w o) -> p w o", o=1).broadcast_to(
                    [P, W, P])
                mb_xb = mb_x[:].rearrange("p (w o) -> p w o", o=1).broadcast_to(
                    [P, W, P])
                nc.vector.tensor_tensor(out=xres[:], in0=xf_s[:], in1=rs_xb,
                                        op=OP.mult)
                nc.vector.tensor_tensor(out=xres[:], in0=xres[:], in1=mb_xb,
                                        op=OP.add)
                if flags["gnb"]:
                    g_b = bcast_t["gnb"][:].rearrange(
                        "p (o f) -> p o f", o=1).broadcast_to([P, W, P])
                    nc.vector.tensor_tensor(out=xres[:], in0=xres[:], in1=g_b,
                                            op=OP.mult)
                if flags["bnb"]:
                    b_b = bcast_t["bnb"][:].rearrange(
                        "p (o f) -> p o f", o=1).broadcast_to([P, W, P])
                    nc.vector.tensor_tensor(out=xres[:], in0=xres[:], in1=b_b,
                                            op=OP.add)

                out_sb = wp.tile([P, W, P], f32, tag="out_sb")
                nc.vector.tensor_tensor(out=out_sb[:], in0=po[:], in1=xres[:],
                                        op=OP.add)
                nc.sync.dma_start(
                    out=out_d[w0 * P: w0 * P + R, :].rearrange(
                        "(w p) f -> p w f", p=P),
                    in_=out_sb[:])

                stream_col += sup_slots // 16
                tile_col += W * nt

    nc.compile()
    return nc


def _ensure_ntff_hook():
    """Register the axon NTFF profile hook when the container's antenv
    package lacks axon_hooks (needed for trace=True under axon)."""
    import types
    try:
        from antenv.axon_hooks import get_axon_ntff_profile_hook  # noqa: F401
        return
    except ImportError:
        pass
    if "/root/.axon_site" not in sys.path:
        sys.path.insert(0, "/root/.axon_site")
    from trn_agent_boot.trn_boot import _ntff_profile_via_ctypes
    import antenv
    hook = _ntff_profile_via_ctypes("/opt/axon/libaxon_pjrt.so")
    mod = types.ModuleType("antenv.axon_hooks")
    mod.get_axon_ntff_profile_hook = lambda: hook
    mod.set_axon_ntff_profile_hook = lambda h: None
    sys.modules["antenv.axon_hooks"] = mod
    antenv.axon_hooks = mod


# ----------------------------------------------------------------------------
# entry point
# ----------------------------------------------------------------------------

def kernel(x, edge_index, degree, Wr, br, Wb1, bb1, Wb2, bb2, g_rb, b_rb,
           W1, b1, W2, b2, g_n, b_n):
    x = np.asarray(x, dtype=np.float32)
    edge_index = np.asarray(edge_index)
    degree = np.asarray(degree, dtype=np.float32)
    N, D = x.shape
    H = np.asarray(Wb1).shape[1]

    plan = _plan(x, edge_index, degree)
    spc, n_win = plan["spc"], plan["n_win"]
    n_slots = spc * N_CORES
    node_of_slot = plan["node_of_slot"]

    # permuted node data (pad with zeros)
    x_pad = np.zeros((n_slots, D), np.float32)
    x_pad[: N] = x
    deg_pad = np.zeros(n_slots, np.float32)
    deg_pad[: N] = degree
    x_perm = x_pad[node_of_slot]          # [n_slots, D] rows in slot order
    deg_perm = deg_pad[node_of_slot]

    x_bf = x.astype(ml_dtypes.bfloat16)

    flags = dict(
        H=H,
        b1nz=bool(np.any(np.asarray(b1) != 0)),
        brb=bool(np.any(np.asarray(br) != 0)),
        bb2b=bool(np.any(np.asarray(bb2) != 0)),
        grbb=bool(np.any(np.asarray(g_rb) != 1)),
        brbb=bool(np.any(np.asarray(b_rb) != 0)),
        gnb=bool(np.any(np.asarray(g_n) != 1)),
        bnb=bool(np.any((np.asarray(b_n) + np.asarray(b2)) != 0)),
    )

    nc = _build_program(plan, flags)

    nH = H // P
    iota_arr = np.broadcast_to(np.arange(P, dtype=np.uint8)[None, :], (P, P)).copy()
    ident = np.eye(P, dtype=ml_dtypes.bfloat16)
    Wb2c = np.asarray(Wb2, np.float32).reshape(nH, P, P).transpose(1, 0, 2)
    W2c = np.asarray(W2, np.float32).reshape(nH, P, P).transpose(1, 0, 2)
    shared = {
        "xg": x_bf,
        "iota": iota_arr,
        "ident": ident,
        "Wr": np.asarray(Wr, np.float32).astype(ml_dtypes.bfloat16),
        "Wb1": np.asarray(Wb1, np.float32).astype(ml_dtypes.bfloat16),
        "W1": np.asarray(W1, np.float32).astype(ml_dtypes.bfloat16),
        "Wb2c": Wb2c.astype(ml_dtypes.bfloat16),
        "W2c": W2c.astype(ml_dtypes.bfloat16),
        "bb1c": np.asarray(bb1, np.float32).reshape(nH, P).T.copy(),
        "b1c": np.asarray(b1, np.float32).reshape(nH, P).T.copy(),
    }
    if flags["brb"]:
        shared["brb"] = np.broadcast_to(np.asarray(br, np.float32)[None, :], (P, P)).copy()
    if flags["bb2b"]:
        shared["bb2b"] = np.broadcast_to(np.asarray(bb2, np.float32)[None, :], (P, P)).copy()
    if flags["grbb"]:
        shared["grbb"] = np.broadcast_to(np.asarray(g_rb, np.float32)[None, :], (P, P)).copy()
    if flags["brbb"]:
        shared["brbb"] = np.broadcast_to(np.asarray(b_rb, np.float32)[None, :], (P, P)).copy()
    if flags["gnb"]:
        shared["gnb"] = np.broadcast_to(np.asarray(g_n, np.float32)[None, :], (P, P)).copy()
    if flags["bnb"]:
        shared["bnb"] = np.broadcast_to(
            (np.asarray(b_n, np.float32) + np.asarray(b2, np.float32))[None, :],
            (P, P)).copy()

    in_maps = []
    for c in range(N_CORES):
        xc = x_perm[c * spc:(c + 1) * spc]
        m = dict(shared)
        m["xt"] = np.ascontiguousarray(xc.T).astype(ml_dtypes.bfloat16)
        m["xf"] = xc
        m["deg"] = np.ascontiguousarray(
            deg_perm[c * spc:(c + 1) * spc].reshape(n_win, P).T)
        m["idx"] = np.tile(
            plan["idx_streams"][c].reshape(-1, 16).T, (8, 1)).copy()
        m["slots"] = np.ascontiguousarray(plan["slots_2d"][c].T)
        in_maps.append(m)

    global LAST_EXEC_TIME_NS
    if TRACE:
        _ensure_ntff_hook()
    res = run_bass_kernel_spmd(nc, in_maps, list(range(N_CORES)), trace=TRACE)
    LAST_EXEC_TIME_NS = res.exec_time_ns
    out_slots = np.concatenate([np.asarray(res.results[c]["out"])
                                for c in range(N_CORES)], axis=0)
    out = out_slots[plan["slot_of_node"][:N]]
    return out.astype(np.float32)


# revision 53
# speedup vs baseline: 1.2051x; 1.2051x over previous
"""Trainium2 Bass kernel for BoundaryConvLayer GNN message passing.

Strategy (8 NeuronCores, no collectives):
  - Nodes are assigned to 8 cores x (windows of 128 node slots), load-balanced
    by in-degree so every window has ~equal incoming-edge count.
  - x is replicated (bf16) on every core as the gather table; each core
    DMA-gathers x[src] rows (256B) for the edges of its own windows and
    reduces them on the TensorEngine: agg_window += S_tile.T @ G_tile where
    S is a one-hot (edge -> dst slot) matrix built on the VectorEngine with
    an is_equal compare against an iota constant.
  - dma_gather descriptor generation is round-robined over the 4 SWDGE
    queues (queue q runs on Q7 core pair q) to parallelize the Q7 work.
  - The per-node MLPs (rate / rob_bound / fc) run in bf16 with
    transposed-activation matmul patterns; LayerNorms via bn_stats in f32.
  - Output is written per-core and inverse-permuted on the host.
"""

import sys

sys.path.insert(0, "/opt/trn_rl_repo")

import heapq

import ml_dtypes
import numpy as np

import bass_rust as _bass_rust

from concourse import bacc, bass, tile
from concourse.bass_utils import run_bass_kernel_spmd
from concourse.hw_specs import get_activation_tables

mybir = bass.mybir
f32 = mybir.dt.float32
bf16 = mybir.dt.bfloat16
u8 = mybir.dt.uint8
i16 = mybir.dt.int16

P = 128
N_CORES = 8
TRACE = False
LAST_EXEC_TIME_NS = None
# gather-index chunk boundaries (int16 idx limit 32767 per chunk); sized so
# the per-(window,chunk) edge-count maxima quantize to fewer 128-edge tiles
CHUNK_BOUNDS = [0, 27000, 54000, 81000, 100000]
N_CHUNKS = 4
STRIP_TRAILING_PADS = False
EPS = 1e-4
LN_EPS = 1e-5


# ----------------------------------------------------------------------------
# host-side planning
# ----------------------------------------------------------------------------

def _balanced_assignment(indeg, n_slots):
    """Assign node ids (len(indeg) <= n_slots) to n_slots//128 windows of 128
    slots each, minimizing max window edge-load. Returns slot_of_node."""
    n_win = n_slots // P
    n = len(indeg)
    order = np.argsort(-indeg, kind="stable")
    slot_of_node = np.empty(n_slots, dtype=np.int64)
    # heap of (load, win); counts per window
    heap = [(0.0, w) for w in range(n_win)]
    heapq.heapify(heap)
    counts = np.zeros(n_win, dtype=np.int64)
    # assign real nodes by descending degree, then dummies
    ids = np.concatenate([order, np.arange(n, n_slots)])
    degs = np.concatenate([indeg[order], np.zeros(n_slots - n, dtype=indeg.dtype)])
    for i in range(n_slots):
        while True:
            load, w = heapq.heappop(heap)
            if counts[w] < P:
                break
        slot_of_node[ids[i]] = w * P + counts[w]
        counts[w] += 1
        heapq.heappush(heap, (load + float(degs[i]), w))
    return slot_of_node


def _plan(x, edge_index, degree):
    N, D = x.shape
    assert D == P
    E = edge_index.shape[1]
    spc = -(-N // (N_CORES * P)) * P          # node slots per core
    n_win = spc // P                           # windows per core
    n_slots = spc * N_CORES
    n_chunks = N_CHUNKS
    bounds = np.array(CHUNK_BOUNDS, dtype=np.int64)
    assert bounds[-1] >= N and (np.diff(bounds) <= 32768).all()

    src = np.asarray(edge_index[0], dtype=np.int64)
    dst = np.asarray(edge_index[1], dtype=np.int64)
    indeg = np.bincount(dst, minlength=N)
    slot_of_node = _balanced_assignment(indeg, n_slots)
    node_of_slot = np.empty(n_slots, dtype=np.int64)
    node_of_slot[slot_of_node] = np.arange(n_slots)

    gslot = slot_of_node[dst]
    core = gslot // spc
    pos = gslot % spc
    gw = pos >> 7                    # window within core
    lane = pos & 127                 # dst slot within window
    chunk = np.searchsorted(bounds, src, side="right") - 1
    srcloc = (src - bounds[chunk]).astype(np.int16)

    # group edges by (core, gw, chunk); compute per-group ranks
    key = (core * n_win + gw) * n_chunks + chunk
    sort_idx = np.argsort(key, kind="stable")
    key_s = key[sort_idx]
    first = np.ones(E, dtype=bool)
    first[1:] = key_s[1:] != key_s[:-1]
    starts = np.flatnonzero(first)
    group_of = np.cumsum(first) - 1
    rank = np.arange(E) - starts[group_of]

    counts = np.bincount(key, minlength=N_CORES * n_win * n_chunks)
    counts = counts.reshape(N_CORES, n_win, n_chunks)
    # tiles per chunk (shared across cores/windows so the program is SPMD)
    TC = np.maximum(1, -(-counts.max(axis=(0, 1)) // P))   # [n_chunks]
    nt = int(TC.sum())
    cumTC = np.concatenate([[0], np.cumsum(TC)])           # [n_chunks+1]

    # super-tile structure: groups of up to 4 windows
    supers = []
    w0 = 0
    while w0 < n_win:
        W = min(4, n_win - w0)
        supers.append((w0, W))
        w0 += W
    super_of_win = np.zeros(n_win, dtype=np.int64)
    Wdim = np.zeros(len(supers), dtype=np.int64)
    super_base = np.zeros(len(supers), dtype=np.int64)   # slot offset in stream
    off = 0
    for si, (sw0, W) in enumerate(supers):
        super_of_win[sw0:sw0 + W] = si
        Wdim[si] = W
        super_base[si] = off
        off += W * nt * P
    total_slots = off
    assert total_slots == n_win * nt * P
    n_tiles_tot = n_win * nt

    # per-edge stream position (within its core's stream)
    # layout within a super: [chunk c: [window wl: TC[c]*128 slots]]
    e_si = super_of_win[gw]
    e_wl = gw - np.array([s[0] for s in supers])[e_si]
    e_W = Wdim[e_si]
    e_pos = (super_base[e_si]
             + e_W * P * cumTC[chunk]
             + e_wl * int(P) * TC[chunk])
    e_pos_sorted = e_pos[sort_idx] + rank
    assert (rank < TC[chunk[sort_idx]] * P).all(), "tile capacity exceeded"

    # build per-core streams
    idx_streams = np.zeros((N_CORES, total_slots), dtype=np.int16)
    slots_2d = np.full((N_CORES, n_tiles_tot, P), 255, dtype=np.uint8)
    core_s = core[sort_idx]
    srcloc_s = srcloc[sort_idx]
    lane_s = lane[sort_idx]
    gw_s = gw[sort_idx]
    chunk_s = chunk[sort_idx]
    rank_s = rank
    e_wl_s = e_wl[sort_idx]
    e_W_s = e_W[sort_idx]
    for c in range(N_CORES):
        m = core_s == c
        idx_streams[c, e_pos_sorted[m]] = srcloc_s[m]
        # chunk-major tile columns within each super: the device builds one
        # one-hot S per chunk from a contiguous slots_t slice
        stb = (gw_s[m] - e_wl_s[m]) * nt
        col = (stb + cumTC[chunk_s[m]] * e_W_s[m]
               + e_wl_s[m] * TC[chunk_s[m]] + (rank_s[m] >> 7))
        slots_2d[c, col, rank_s[m] & 127] = lane_s[m]
        if STRIP_TRAILING_PADS:
            # trailing pad slots of each (super, chunk) gather call -> -1 so
            # the Q7 ucode's trailing-negative strip skips their descriptors
            used = np.zeros(total_slots + 1, dtype=bool)
            used[e_pos_sorted[m]] = True
            for si in range(len(supers)):
                W = int(Wdim[si])
                for ch in range(n_chunks):
                    b0 = int(super_base[si]) + W * P * cumTC[ch]
                    b1 = b0 + W * P * (cumTC[ch + 1] - cumTC[ch])
                    blk = used[b0:b1]
                    nz = np.flatnonzero(blk)
                    last = (nz[-1] + 1) if len(nz) else 0
                    idx_streams[c, b0 + last:b1] = -1

    plan = dict(
        N=N, D=D, E=E, spc=spc, n_win=n_win, n_chunks=n_chunks,
        TC=[int(t) for t in TC], nt=nt, cumTC=[int(t) for t in cumTC],
        supers=supers, total_slots=total_slots, n_tiles_tot=n_tiles_tot,
        slot_of_node=slot_of_node, node_of_slot=node_of_slot,
        idx_streams=idx_streams, slots_2d=slots_2d,
    )
    return plan


# ----------------------------------------------------------------------------
# device program
# ----------------------------------------------------------------------------

class _Bacc(bacc.Bacc):
    """Bacc with an activation-table chooser biased to the exp+ln+identity
    table (act_info idx 6) so exp/ln pairs don't thrash ACT_TABLE_LOADs.
    The per-instruction ids are positions in the passed list, so tables are
    emptied (not removed) to keep ids aligned with act_info.json."""

    def insert_act_table_loads(self):
        has_activation = any(
            isinstance(i, mybir.InstActivation)
            for b in self.main_func.blocks
            for i in b.instructions
        )
        if not has_activation:
            return
        tables = list(get_activation_tables(self.m.arch).items())
        tables = [(n, (s if i >= 6 else set())) for i, (n, s) in enumerate(tables)]
        _bass_rust.insert_act_table_loads(self, tables)


def _build_program(plan, flags):
    n_win, n_chunks = plan["n_win"], plan["n_chunks"]
    TC, nt, cumTC = plan["TC"], plan["nt"], plan["cumTC"]
    supers = plan["supers"]
    spc = plan["spc"]
    N = plan["N"]
    total_slots = plan["total_slots"]
    n_tiles_tot = plan["n_tiles_tot"]
    H = flags["H"]
    nH = H // P

    nc = _Bacc("TRN2", target_bir_lowering=False, num_swdge_queues=4)

    xg_d = nc.dram_tensor("xg", [N, P], bf16, kind="ExternalInput")
    xt_d = nc.dram_tensor("xt", [P, spc], bf16, kind="ExternalInput")
    xf_d = nc.dram_tensor("xf", [spc, P], f32, kind="ExternalInput")
    deg_d = nc.dram_tensor("deg", [P, n_win], f32, kind="ExternalInput")
    idx_d = nc.dram_tensor("idx", [P, total_slots // 16], i16, kind="ExternalInput")
    slots_d = nc.dram_tensor("slots", [P, n_tiles_tot], u8, kind="ExternalInput")
    iota_d = nc.dram_tensor("iota", [P, P], u8, kind="ExternalInput")
    Wr_d = nc.dram_tensor("Wr", [P, P], bf16, kind="ExternalInput")
    Wb1_d = nc.dram_tensor("Wb1", [P, H], bf16, kind="ExternalInput")
    W1_d = nc.dram_tensor("W1", [P, H], bf16, kind="ExternalInput")
    Wb2c_d = nc.dram_tensor("Wb2c", [P, nH, P], bf16, kind="ExternalInput")
    W2c_d = nc.dram_tensor("W2c", [P, nH, P], bf16, kind="ExternalInput")
    bb1c_d = nc.dram_tensor("bb1c", [P, nH], f32, kind="ExternalInput")
    b1c_d = nc.dram_tensor("b1c", [P, nH], f32, kind="ExternalInput")
    ident_d = nc.dram_tensor("ident", [P, P], bf16, kind="ExternalInput")
    bcast_names = [n for n in ("brb", "bb2b", "grbb", "brbb", "gnb", "bnb")
                   if flags[n]]
    bcast_d = {n: nc.dram_tensor(n, [P, P], f32, kind="ExternalInput")
               for n in bcast_names}
    out_d = nc.dram_tensor("out", [spc, P], f32, kind="ExternalOutput")

    AT = mybir.ActivationFunctionType
    OP = mybir.AluOpType

    with tile.TileContext(nc) as tc:
        with tc.tile_pool(name="const", bufs=1) as cp, \
             tc.tile_pool(name="stream", bufs=3) as sp, \
             tc.tile_pool(name="gat", bufs=3) as gp, \
             tc.tile_pool(name="sS", bufs=2) as ssp, \
             tc.tile_pool(name="work", bufs=2) as wp, \
             tc.tile_pool(name="tiny", bufs=6) as tp, \
             tc.tile_pool(name="ps_big", bufs=2, space="PSUM") as ps_big, \
             tc.tile_pool(name="ps_z", bufs=1, space="PSUM") as ps_z, \
             tc.tile_pool(name="ps_agg", bufs=2, space="PSUM") as ps_agg, \
             tc.tile_pool(name="ps_r", bufs=1, space="PSUM") as ps_r, \
             tc.tile_pool(name="ps_ht", bufs=1, space="PSUM") as ps_ht, \
             tc.tile_pool(name="ps_out", bufs=1, space="PSUM") as ps_out:

            # ---- constants
            def cload(dram, shape, dtype, tag):
                t = cp.tile(shape, dtype, tag=tag)
                nc.sync.dma_start(out=t[:], in_=dram[:])
                return t

            eps_t = cp.tile([P, 1], f32, tag="eps")
            nc.vector.memset(eps_t[:], LN_EPS)
            one_t = cp.tile([P, 1], f32, tag="one")
            nc.vector.memset(one_t[:], 1.0)
            onep_t = cp.tile([P, 1], f32, tag="onep")
            nc.vector.memset(onep_t[:], 1.0 + EPS)

            def bc1(t, shape):
                return t[:].rearrange("p (o f) -> p o f", o=1, f=1).broadcast_to(shape)

            Wr_t = cload(Wr_d, [P, P], bf16, "Wr")
            Wb1_t = cload(Wb1_d, [P, H], bf16, "Wb1")
            W1_t = cload(W1_d, [P, H], bf16, "W1")
            Wb2c_t = cload(Wb2c_d, [P, nH, P], bf16, "Wb2c")
            W2c_t = cload(W2c_d, [P, nH, P], bf16, "W2c")
            bb1c_t = cload(bb1c_d, [P, nH], f32, "bb1c")
            b1c_t = cload(b1c_d, [P, nH], f32, "b1c")
            ident_t = cload(ident_d, [P, P], bf16, "ident")
            iota_t = cload(iota_d, [P, P], u8, "iota")
            deg_t = cload(deg_d, [P, n_win], f32, "deg")
            bcast_t = {n: cload(bcast_d[n], [P, P], f32, n) for n in bcast_names}

            stream_col = 0      # idx_d column offset (int16 cols)
            tile_col = 0        # slots_d column offset
            for si, (w0, W) in enumerate(supers):
                R = W * P
                sup_slots = W * nt * P         # gather slots this super
                # ---- streams
                xt_s = sp.tile([P, R], bf16, tag="xt")
                nc.sync.dma_start(out=xt_s[:], in_=xt_d[:, w0 * P: w0 * P + R])
                xf_s = sp.tile([P, W, P], f32, tag="xf")
                nc.sync.dma_start(
                    out=xf_s[:],
                    in_=xf_d[w0 * P: w0 * P + R, :].rearrange(
                        "(w p) f -> p w f", p=P))
                idx_t = sp.tile([P, sup_slots // 16], i16, tag="idx")
                nc.sync.dma_start(
                    out=idx_t[:],
                    in_=idx_d[:, stream_col: stream_col + sup_slots // 16])
                slots_t = sp.tile([P, W * nt], u8, tag="slots")
                nc.sync.dma_start(
                    out=slots_t[:],
                    in_=slots_d[:, tile_col: tile_col + W * nt])

                # ---- gathers (one per chunk; round-robin over SWDGE queues)
                Gs = []
                Ss = []
                for c in range(n_chunks):
                    cap_c = W * TC[c] * P
                    G = gp.tile([P, W * TC[c], P], bf16, tag=f"G{c}")
                    lo = CHUNK_BOUNDS[c]
                    hi = min(N, CHUNK_BOUNDS[c + 1])
                    base = W * P * cumTC[c]
                    nc.gpsimd.dma_gather(
                        out_ap=G[:],
                        in_ap=xg_d[lo:hi, :],
                        idxs_ap=idx_t[:, base // 16: (base + cap_c) // 16],
                        num_idxs=cap_c, num_idxs_reg=cap_c, elem_size=P,
                        single_packet=False, queue_num=c % 4)
                    Gs.append(G)
                    # one-hot S for this chunk (contiguous slots_t slice)
                    S = ssp.tile([P, W * TC[c], P], bf16, tag=f"S{c}")
                    sb = slots_t[:, W * cumTC[c]: W * cumTC[c + 1]
                                 ].broadcast_to([P, W * TC[c], P])
                    ib = iota_t[:].rearrange(
                        "p (o f) -> p o f", o=1).broadcast_to(
                            [P, W * TC[c], P])
                    nc.vector.tensor_tensor(out=S[:], in0=sb, in1=ib,
                                            op=OP.is_equal)
                    Ss.append(S)

                # ---- rob_bound layer 1 (A-pattern):
                # g1 = softplus(u + bb1) = ln(exp(u + bb1) + 1); exp and ln
                # live in the same HW activation table (no table swaps).
                g1T = wp.tile([P, nH, R], bf16, tag="g1T")
                for cH in range(nH):
                    pb = ps_big.tile([P, R], f32, tag="bigA")
                    nc.tensor.matmul(pb[:], Wb1_t[:, cH * P:(cH + 1) * P],
                                     xt_s[:], start=True, stop=True)
                    t1 = wp.tile([P, R], f32, tag="sp1")
                    nc.scalar.activation(t1[:], pb[:], AT.Exp,
                                         bias=bb1c_t[:, cH:cH + 1], scale=1.0)
                    nc.scalar.activation(g1T[:, cH, :], t1[:], AT.Ln,
                                         bias=1.0, scale=1.0)

                # ---- rob_bound layer 2 (B-pattern) -> psum_z[rows, feat]
                pz = ps_z.tile([P, W, P], f32, tag="z")
                for wl in range(W):
                    for cH in range(nH):
                        nc.tensor.matmul(
                            pz[:, wl, :],
                            g1T[:, cH, wl * P:(wl + 1) * P],
                            Wb2c_t[:, cH, :],
                            start=(cH == 0), stop=(cH == nH - 1))

                # optional bb2: LN input z'' = z + bb2
                z_in = pz
                if flags["bb2b"]:
                    z_sb = wp.tile([P, W, P], f32, tag="z_sb")
                    bb2_b = bcast_t["bb2b"][:].rearrange(
                        "p (o f) -> p o f", o=1).broadcast_to([P, W, P])
                    nc.vector.tensor_tensor(out=z_sb[:], in0=pz[:], in1=bb2_b,
                                            op=OP.add)
                    z_in = z_sb

                # ---- LN(gamma) on z  (batched stats; vector apply)
                mv_z = tp.tile([P, W, 2], f32, tag="mv_z")
                for wl in range(W):
                    stats_z = tp.tile([P, 6], f32, tag="stats_z")
                    nc.vector.bn_stats(stats_z[:], z_in[:, wl, :])
                    nc.vector.bn_aggr(mv_z[:, wl, :], stats_z[:])
                # rsqrt(v + eps) = exp(-0.5 * ln(v + eps)) -- same act table
                lnv_z = tp.tile([P, W], f32, tag="lnv_z")
                nc.scalar.activation(
                    lnv_z[:], mv_z[:, :, 1:2].rearrange("p w o -> p (w o)"),
                    AT.Ln, bias=eps_t[:, 0:1], scale=1.0)
                rs_z = tp.tile([P, W], f32, tag="rs_z")
                nc.scalar.activation(rs_z[:], lnv_z[:], AT.Exp,
                                     bias=0.0, scale=-0.5)
                mb_z = tp.tile([P, W], f32, tag="mb_z")
                nc.vector.scalar_tensor_tensor(
                    out=mb_z[:], in0=mv_z[:, :, 0:1].rearrange("p w o -> p (w o)"),
                    scalar=-1.0, in1=rs_z[:], op0=OP.mult, op1=OP.mult)
                gamma = wp.tile([P, W, P], f32, tag="gamma")
                rs_zb = rs_z[:].rearrange("p (w o) -> p w o", o=1).broadcast_to(
                    [P, W, P])
                mb_zb = mb_z[:].rearrange("p (w o) -> p w o", o=1).broadcast_to(
                    [P, W, P])
                nc.vector.tensor_tensor(out=gamma[:], in0=z_in[:], in1=rs_zb,
                                        op=OP.mult)
                nc.vector.tensor_tensor(out=gamma[:], in0=gamma[:], in1=mb_zb,
                                        op=OP.add)
                if flags["grbb"]:
                    g_b = bcast_t["grbb"][:].rearrange(
                        "p (o f) -> p o f", o=1).broadcast_to([P, W, P])
                    nc.vector.tensor_tensor(out=gamma[:], in0=gamma[:], in1=g_b,
                                            op=OP.mult)
                if flags["brbb"]:
                    b_b = bcast_t["brbb"][:].rearrange(
                        "p (o f) -> p o f", o=1).broadcast_to([P, W, P])
                    nc.vector.tensor_tensor(out=gamma[:], in0=gamma[:], in1=b_b,
                                            op=OP.add)

                # ---- rate = softplus(x @ Wr + br)   [rows, feat]
                pr = ps_r.tile([P, W, P], f32, tag="r")
                for wl in range(W):
                    nc.tensor.matmul(pr[:, wl, :], xt_s[:, wl * P:(wl + 1) * P],
                                     Wr_t[:], start=True, stop=True)
                rate = wp.tile([P, W, P], f32, tag="rate")
                if flags["brb"]:
                    br_b = bcast_t["brb"][:].rearrange(
                        "p (o f) -> p o f", o=1).broadcast_to([P, W, P])
                    nc.vector.tensor_tensor(out=rate[:], in0=pr[:], in1=br_b,
                                            op=OP.add)
                    nc.scalar.activation(rate[:], rate[:], AT.Exp,
                                         bias=0.0, scale=1.0)
                else:
                    nc.scalar.activation(rate[:], pr[:], AT.Exp,
                                         bias=0.0, scale=1.0)
                nc.scalar.activation(rate[:], rate[:], AT.Ln,
                                     bias=1.0, scale=1.0)

                # ---- agg: one-hot matmul accumulation per window
                pagg = ps_agg.tile([P, W, P], f32, tag="agg")
                for wl in range(W):
                    k = 0
                    for c in range(n_chunks):
                        for j in range(TC[c]):
                            nc.tensor.matmul(
                                pagg[:, wl, :],
                                Ss[c][:, wl * TC[c] + j, :],
                                Gs[c][:, wl * TC[c] + j, :],
                                start=(k == 0), stop=(k == nt - 1))
                            k += 1

                # ---- h = (rate*agg + gamma) / (1 + rate*deg + EPS)
                num = wp.tile([P, W, P], f32, tag="num")
                nc.vector.tensor_tensor(out=num[:], in0=rate[:], in1=pagg[:],
                                        op=OP.mult)
                nc.vector.tensor_tensor(out=num[:], in0=num[:], in1=gamma[:],
                                        op=OP.add)
                den = wp.tile([P, W, P], f32, tag="den")
                deg_b = deg_t[:, w0:w0 + W].rearrange(
                    "p (w o) -> p w o", o=1).broadcast_to([P, W, P])
                nc.vector.tensor_tensor(out=den[:], in0=rate[:], in1=deg_b,
                                        op=OP.mult)
                nc.vector.tensor_tensor(out=den[:], in0=den[:],
                                        in1=bc1(onep_t, [P, W, P]), op=OP.add)
                rcp = wp.tile([P, W, P], f32, tag="rcp")
                nc.vector.reciprocal_approx_fast(out=rcp[:], in_=den[:])
                h_bf = wp.tile([P, W, P], bf16, tag="h_bf")
                nc.vector.tensor_tensor(out=h_bf[:], in0=num[:], in1=rcp[:],
                                        op=OP.mult)

                # ---- hT via PE transpose
                pht = ps_ht.tile([P, W, P], bf16, tag="ht")
                for wl in range(W):
                    nc.tensor.transpose(pht[:, wl, :], h_bf[:, wl, :], ident_t[:])
                hT = wp.tile([P, R], bf16, tag="hT")
                nc.scalar.activation(hT[:], pht[:].rearrange("p w f -> p (w f)"),
                                     AT.Identity, bias=0.0, scale=1.0)

                # ---- fc layer 1 (A-pattern on hT) + gelu
                g2T = wp.tile([P, nH, R], bf16, tag="g2T")
                for cH in range(nH):
                    pf = ps_big.tile([P, R], f32, tag="bigA")
                    nc.tensor.matmul(pf[:], W1_t[:, cH * P:(cH + 1) * P],
                                     hT[:], start=True, stop=True)
                    nc.scalar.activation(g2T[:, cH, :], pf[:], AT.Gelu,
                                         bias=b1c_t[:, cH:cH + 1], scale=1.0)

                # ---- fc layer 2 (B-pattern) -> psum_out[rows, feat]
                po = ps_out.tile([P, W, P], f32, tag="o")
                for wl in range(W):
                    for cH in range(nH):
                        nc.tensor.matmul(
                            po[:, wl, :],
                            g2T[:, cH, wl * P:(wl + 1) * P],
                            W2c_t[:, cH, :],
                            start=(cH == 0), stop=(cH == nH - 1))

                # ---- x_res = LN(x) (f32) ; out = psum_out + x_res (+ bnb)
                mv_x = tp.tile([P, W, 2], f32, tag="mv_x")
                for wl in range(W):
                    stats_x = tp.tile([P, 6], f32, tag="stats_x")
                    nc.vector.bn_stats(stats_x[:], xf_s[:, wl, :])
                    nc.vector.bn_aggr(mv_x[:, wl, :], stats_x[:])
                lnv_x = tp.tile([P, W], f32, tag="lnv_x")
                nc.scalar.activation(
                    lnv_x[:], mv_x[:, :, 1:2].rearrange("p w o -> p (w o)"),
                    AT.Ln, bias=eps_t[:, 0:1], scale=1.0)
                rs_x = tp.tile([P, W], f32, tag="rs_x")
                nc.scalar.activation(rs_x[:], lnv_x[:], AT.Exp,
                                     bias=0.0, scale=-0.5)
                mb_x = tp.tile([P, W], f32, tag="mb_x")
                nc.vector.scalar_tensor_tensor(
                    out=mb_x[:], in0=mv_x[:, :, 0:1].rearrange("p w o -> p (w o)"),
                    scalar=-1.0, in1=rs_x[:], op0=OP.mult, op1=OP.mult)
                xres = wp.tile([P, W, P], f32, tag="xres")
                rs_xb = rs_x[:].rearrange("p (w o) -> p w o", o=1).broadcast_to(
                    [P, W, P])
                mb_xb = mb_x[:].rearrange("p (w o) -> p w o", o=1).broadcast_to(
                    [P, W, P])
                nc.vector.tensor_tensor(out=xres[:], in0=xf_s[:], in1=rs_xb,
                                        op=OP.mult)
                nc.vector.tensor_tensor(out=xres[:], in0=xres[:], in1=mb_xb,
                                        op=OP.add)
                if flags["gnb"]:
                    g_b = bcast_t["gnb"][:].rearrange(
                        "p (o f) -> p o f", o=1).broadcast_to([P, W, P])
                    nc.vector.tensor_tensor(out=xres[:], in0=xres[:], in1=g_b,
                                            op=OP.mult)
                if flags["bnb"]:
                    b_b = bcast_t["bnb"][:].rearrange(
                        "p (o f) -> p o f", o=1).broadcast_to([P, W, P])
                    nc.vector.tensor_tensor(out=xres[:], in0=xres[:], in1=b_b,
                                            op=OP.add)

                out_sb = wp.tile([P, W, P], f32, tag="out_sb")
                nc.vector.tensor_tensor(out=out_sb[:], in0=po[:], in1=xres[:],
                                        op=OP.add)
                nc.sync.dma_start(
                    out=out_d[w0 * P: w0 * P + R, :].rearrange(
                        "(w p) f -> p w f", p=P),
                    in_=out_sb[:])

                stream_col += sup_slots // 16
                tile_col += W * nt

    nc.compile()
    return nc


def _ensure_ntff_hook():
    """Register the axon NTFF profile hook when the container's antenv
    package lacks axon_hooks (needed for trace=True under axon)."""
    import types
    try:
        from antenv.axon_hooks import get_axon_ntff_profile_hook  # noqa: F401
        return
    except ImportError:
        pass
    if "/root/.axon_site" not in sys.path:
        sys.path.insert(0, "/root/.axon_site")
    from trn_agent_boot.trn_boot import _ntff_profile_via_ctypes
    import antenv
    hook = _ntff_profile_via_ctypes("/opt/axon/libaxon_pjrt.so")
    mod = types.ModuleType("antenv.axon_hooks")
    mod.get_axon_ntff_profile_hook = lambda: hook
    mod.set_axon_ntff_profile_hook = lambda h: None
    sys.modules["antenv.axon_hooks"] = mod
    antenv.axon_hooks = mod


# ----------------------------------------------------------------------------
# entry point
# ----------------------------------------------------------------------------

def kernel(x, edge_index, degree, Wr, br, Wb1, bb1, Wb2, bb2, g_rb, b_rb,
           W1, b1, W2, b2, g_n, b_n):
    x = np.asarray(x, dtype=np.float32)
    edge_index = np.asarray(edge_index)
    degree = np.asarray(degree, dtype=np.float32)
    N, D = x.shape
    H = np.asarray(Wb1).shape[1]

    plan = _plan(x, edge_index, degree)
    spc, n_win = plan["spc"], plan["n_win"]
    n_slots = spc * N_CORES
    node_of_slot = plan["node_of_slot"]

    # permuted node data (pad with zeros)
    x_pad = np.zeros((n_slots, D), np.float32)
    x_pad[: N] = x
    deg_pad = np.zeros(n_slots, np.float32)
    deg_pad[: N] = degree
    x_perm = x_pad[node_of_slot]          # [n_slots, D] rows in slot order
    deg_perm = deg_pad[node_of_slot]

    x_bf = x.astype(ml_dtypes.bfloat16)

    flags = dict(
        H=H,
        b1nz=bool(np.any(np.asarray(b1) != 0)),
        brb=bool(np.any(np.asarray(br) != 0)),
        bb2b=bool(np.any(np.asarray(bb2) != 0)),
        grbb=bool(np.any(np.asarray(g_rb) != 1)),
        brbb=bool(np.any(np.asarray(b_rb) != 0)),
        gnb=bool(np.any(np.asarray(g_n) != 1)),
        bnb=bool(np.any((np.asarray(b_n) + np.asarray(b2)) != 0)),
    )

    nc = _build_program(plan, flags)

    nH = H // P
    iota_arr = np.broadcast_to(np.arange(P, dtype=np.uint8)[None, :], (P, P)).copy()
    ident = np.eye(P, dtype=ml_dtypes.bfloat16)
    Wb2c = np.asarray(Wb2, np.float32).reshape(nH, P, P).transpose(1, 0, 2)
    W2c = np.asarray(W2, np.float32).reshape(nH, P, P).transpose(1, 0, 2)
    shared = {
        "xg": x_bf,
        "iota": iota_arr,
        "ident": ident,
        "Wr": np.asarray(Wr, np.float32).astype(ml_dtypes.bfloat16),
        "Wb1": np.asarray(Wb1, np.float32).astype(ml_dtypes.bfloat16),
        "W1": np.asarray(W1, np.float32).astype(ml_dtypes.bfloat16),
        "Wb2c": Wb2c.astype(ml_dtypes.bfloat16),
        "W2c": W2c.astype(ml_dtypes.bfloat16),
        "bb1c": np.asarray(bb1, np.float32).reshape(nH, P).T.copy(),
        "b1c": np.asarray(b1, np.float32).reshape(nH, P).T.copy(),
    }
    if flags["brb"]:
        shared["brb"] = np.broadcast_to(np.asarray(br, np.float32)[None, :], (P, P)).copy()
    if flags["bb2b"]:
        shared["bb2b"] = np.broadcast_to(np.asarray(bb2, np.float32)[None, :], (P, P)).copy()
    if flags["grbb"]:
        shared["grbb"] = np.broadcast_to(np.asarray(g_rb, np.float32)[None, :], (P, P)).copy()
    if flags["brbb"]:
        shared["brbb"] = np.broadcast_to(np.asarray(b_rb, np.float32)[None, :], (P, P)).copy()
    if flags["gnb"]:
        shared["gnb"] = np.broadcast_to(np.asarray(g_n, np.float32)[None, :], (P, P)).copy()
    if flags["bnb"]:
        shared["bnb"] = np.broadcast_to(
            (np.asarray(b_n, np.float32) + np.asarray(b2, np.float32))[None, :],
            (P, P)).copy()

    in_maps = []
    for c in range(N_CORES):
        xc = x_perm[c * spc:(c + 1) * spc]
        m = dict(shared)
        m["xt"] = np.ascontiguousarray(xc.T).astype(ml_dtypes.bfloat16)
        m["xf"] = xc
        m["deg"] = np.ascontiguousarray(
            deg_perm[c * spc:(c + 1) * spc].reshape(n_win, P).T)
        m["idx"] = np.tile(
            plan["idx_streams"][c].reshape(-1, 16).T, (8, 1)).copy()
        m["slots"] = np.ascontiguousarray(plan["slots_2d"][c].T)
        in_maps.append(m)

    global LAST_EXEC_TIME_NS
    if TRACE:
        _ensure_ntff_hook()
    res = run_bass_kernel_spmd(nc, in_maps, list(range(N_CORES)), trace=TRACE)
    LAST_EXEC_TIME_NS = res.exec_time_ns
    out_slots = np.concatenate([np.asarray(res.results[c]["out"])
                                for c in range(N_CORES)], axis=0)
    out = out_slots[plan["slot_of_node"][:N]]
    return out.astype(np.float32)
